# revision 33
# baseline (speedup 1.0000x reference)
"""AnchorHead (RetinaNet-style) Trainium2 kernel.

8 NeuronCores, data-parallel over batch (1 image per core), weights
replicated, no collectives. Per core: for each pyramid level (128^2, 64^2,
32^2) and each branch (cls, bbox): 4x [3x3 conv 256->256 + ReLU] then a
final 3x3 conv (720 or 36 channels).

Conv-as-matmul with fp32 PSUM accumulation; operand dtype is bf16
(USE_BF16, rel_err ~5.6e-3) or float32r/TF32 (~5.8e-4), both host-rounded
so DMA needs no on-chip conversion:
 - intermediate convs ("form B"): activations [ci(128 part) x 2, padded
   spatial] as the moving operand; weights [ci,co] stationary; PSUM
   [co, pixels]; 18 accumulating matmuls per tile (9 taps x 2 ci halves);
   row-aligned pixel tiles (N = rows*(W+2)) so a fused bias+ReLU DVE
   epilogue writes only interior columns of the next padded buffer.
 - final convs ("form A"): activations stationary [ci, row of W pixels],
   weights [ci, co] moving; PSUM [pix, co]; outputs DMA out contiguously
   in NHWC.

Level 0 streams row-strips through per-branch DRAM ping-pongs (full-width
strip writes land their zeroed junk columns on the zero borders); levels
1/2 are SBUF-resident. All tile pools are opened concurrently on disjoint
SBUF so the Tile scheduler overlaps branches, levels, and final phases;
weights (intermediate + final) are preloaded on separate DMA rings.
"""
import numpy as np

import concourse.bass as bass
import concourse.tile as tile
from concourse import bacc, mybir
from concourse.bass_utils import run_bass_kernel_spmd

import ml_dtypes

F32 = mybir.dt.float32
F32R = mybir.dt.float32r
BF16 = mybir.dt.bfloat16
USE_BF16 = True
DT_MM = BF16 if USE_BF16 else F32R   # matmul-operand storage dtype
ADD = mybir.AluOpType.add
MAX = mybir.AluOpType.max

LEVELS = [(128, 128), (64, 64), (32, 32)]
C = 256
NCLS = 720
NBOX = 36
STRIPS0 = [12] + [24] * 4 + [20]   # L0 strip heights (sum 128); small lead-in strip lets PE start early


def row_splits(H, W):
    # rows-per-matmul-tile for resident levels; every N = rows*(W+2) >= 256
    if H == 128:
        return [3] * 42 + [2]
    if H == 64:
        return [6] * 10 + [4]
    if H == 32:
        return [11, 11, 10]
    raise ValueError(H)


def strip_row_splits(S):
    sp = [3] * (S // 3)
    if S % 3:
        sp.append(S % 3)  # 2-row tail tile (N=260) for S=20
    return sp


def build_program(levels=(0, 1, 2), branches=(0, 1), nlayers=4, do_final=True):
    nc = bacc.Bacc("TRN2", target_bir_lowering=False, debug=False, num_devices=8)

    xs = []
    for li, (H, W) in enumerate(LEVELS):
        xs.append(nc.declare_dram_parameter(
            f"x{li}", [2, 128, H + 2, W + 2], DT_MM, isOutput=False))
    wmid = nc.declare_dram_parameter("wmid", [144, 128, 256], DT_MM, isOutput=False)
    bmid = nc.declare_dram_parameter("bmid", [16, 128, 1], F32, isOutput=False)
    wcls = nc.declare_dram_parameter("wcls", [18, 128, NCLS], DT_MM, isOutput=False)
    bcls = nc.declare_dram_parameter("bcls", [128, NCLS], F32, isOutput=False)
    wbox = nc.declare_dram_parameter("wbox", [18, 128, NBOX], DT_MM, isOutput=False)
    bbox = nc.declare_dram_parameter("bbox", [128, NBOX], F32, isOutput=False)
    couts, bouts = [], []
    for li, (H, W) in enumerate(LEVELS):
        couts.append(nc.declare_dram_parameter(f"c{li}", [H * W, NCLS], F32, isOutput=True))
        bouts.append(nc.declare_dram_parameter(f"b{li}", [H * W, NBOX], F32, isOutput=True))

    H0, W0 = LEVELS[0]
    acts = [nc.dram_tensor(f"act{i}", [2, 128, H0 + 2, W0 + 2], DT_MM) for i in range(4)]

    with tile.TileContext(nc) as tc:
        _build_tile(tc, nc, xs, wmid, bmid, wcls, bcls, wbox, bbox, couts, bouts,
                    acts, levels, branches, nlayers, do_final)
    nc.compile()
    return nc


def _build_tile(tc, nc, xs, wmid, bmid, wcls, bcls, wbox, bbox, couts, bouts,
                acts, levels, branches, nlayers, do_final):
    from contextlib import ExitStack

    with ExitStack() as g:
        glob = g.enter_context(tc.tile_pool(name="glob", bufs=1))
        ps_mm = g.enter_context(tc.tile_pool(name="ps_mm", bufs=5, space="PSUM"))
        ps_fa = g.enter_context(tc.tile_pool(name="ps_fa", bufs=1, space="PSUM"))
        ps_fb = g.enter_context(tc.tile_pool(name="ps_fb", bufs=1, space="PSUM"))

        bia = glob.tile([128, 16, 1], F32)
        nc.sync.dma_start(bia[:], bmid.rearrange("k p o -> p k o"))
        bclst = glob.tile([128, NCLS], F32)
        nc.sync.dma_start(bclst[:], bcls[:, :])
        bboxt = glob.tile([128, NBOX], F32)
        nc.sync.dma_start(bboxt[:], bbox[:, :])

        if 0 in levels:
            Hp, Wp = LEVELS[0][0] + 2, LEVELS[0][1] + 2
            zt = glob.tile([128, 2, Wp], DT_MM)
            nc.vector.memset((zt.bitcast(F32) if DT_MM == F32R else zt)[:], 0.0)
            # only the top/bottom padded rows need zeroing here: interior rows
            # (including their border columns) are fully written by every
            # layer's full-width strip DMAs
            for a in acts:
                nc.sync.dma_start(a[:, :, 0, :].rearrange("g p w -> p g w"), zt[:])
                nc.scalar.dma_start(a[:, :, Hp - 1, :].rearrange("g p w -> p g w"), zt[:])

        mmctr = [0]

        def conv_tiles_from(nc, src_flat, dst_write, wr, H, W, row_iter):
            Wp = W + 2
            for (lr0, rt) in row_iter:
                for coh in range(2):
                    Npix = rt * Wp
                    p = ps_mm.tile([128, 512], F32, name=f"mm{mmctr[0]}", tag="mm")
                    mmctr[0] += 1
                    k = 0
                    for t in range(9):
                        dy, dx = t // 3, t % 3
                        off = (dy - 1) * Wp + (dx - 1)
                        for gg in range(2):
                            b0 = (lr0 + 1) * Wp + 1 + off
                            nc.tensor.matmul(
                                p[:, :Npix],
                                wr[:, t * 2 + gg, coh * 128:(coh + 1) * 128],
                                src_flat(gg)[:, b0:b0 + Npix],
                                start=(k == 0), stop=(k == 17))
                            k += 1
                    pv = p[:, :Npix].rearrange("p (r w) -> p r w", w=Wp)[:, :, :W]
                    dst_write(coh, lr0, rt, pv)

        def epi(out_ap, pv, bias_ap):
            nc.vector.tensor_scalar(out_ap, pv, bias_ap, 0.0, ADD, MAX)

        finals = {}
        for br in branches:
            finals[br] = ((wcls, bclst, NCLS) if br == 0 else (wbox, bboxt, NBOX))

        # global pools: intermediate weights, final weights (loaded once),
        # output staging. Level pools are opened concurrently (disjoint SBUF)
        # so the scheduler can overlap phases across levels freely.
        wpg = g.enter_context(tc.tile_pool(name="wpg", bufs=2))
        stp = g.enter_context(tc.tile_pool(name="stp", bufs=2))
        wrts = {}
        if do_final:
            for br in branches:
                wfin, _, NOUT = finals[br]
                wfp = g.enter_context(tc.tile_pool(name=f"fwg{NOUT}", bufs=1))
                wrt = wfp.tile([128, 18, NOUT], DT_MM, name=f"fwg{NOUT}t")
                nc.gpsimd.dma_start(wrt[:], wfin[:, :, :].rearrange("k p m -> p k m"))
                wrts[br] = wrt
        pools = {}
        if 0 in levels:
            pools["sp0"] = g.enter_context(tc.tile_pool(name="sp0", bufs=2))
            pools["op0"] = g.enter_context(tc.tile_pool(name="op0", bufs=2))
        for li in levels:
            if li != 0:
                pools[f"ab{li}"] = g.enter_context(
                    tc.tile_pool(name=f"ab{LEVELS[li][1]}", bufs=1))

        for li in levels:
            H, W = LEVELS[li]
            if li == 0:
                _level0_stream(tc, nc, xs[0], acts, wmid, bia, finals,
                               couts[0], bouts[0], branches, H, W, nlayers,
                               do_final, conv_tiles_from, epi, ps_fa, ps_fb,
                               pools["sp0"], pools["op0"], wpg, wrts, stp)
            else:
                _level_resident(tc, nc, xs[li], wmid, bia, finals,
                                couts[li], bouts[li], branches, H, W, nlayers,
                                do_final, conv_tiles_from, epi, ps_fa, ps_fb,
                                pools[f"ab{li}"], wpg, wrts, stp,
                                seq=(li == 2))


def _load_wmid(nc, wp, wmid, br, layer):
    wr = wp.tile([128, 18, 256], DT_MM, name=f"w{br}{layer}_{_finctr[0]}", tag="wmid")
    i0 = (br * 4 + layer) * 18
    nc.gpsimd.dma_start(wr[:], wmid[i0:i0 + 18].rearrange("k p m -> p k m"))
    return wr


def _level_resident(tc, nc, x, wmid, bia, finals, cout_c, cout_b, branches,
                    H, W, nlayers, do_final, conv_tiles_from, epi, ps_fa, ps_fb,
                    ab, wp, wrts, stp, seq=False):
    """Fully SBUF-resident level.

    seq=False: each branch gets its own buffer pair; layers interleaved
    across branches so the scheduler always has an independent stream.
    seq=True (level 0, buffers too big for two pairs): one shared pair,
    branches processed sequentially."""
    Hp, Wp = H + 2, W + 2
    L = Hp * Wp
    if True:
        if seq:
            pair = [ab.tile([128, 2, L + 2], DT_MM, name=f"bA{W}"),
                    ab.tile([128, 2, L + 2], DT_MM, name=f"bB{W}")]
            bufs = {br: pair for br in branches}
        else:
            bufs = {br: [ab.tile([128, 2, L + 2], DT_MM, name=f"bA{br}{W}"),
                         ab.tile([128, 2, L + 2], DT_MM, name=f"bB{br}{W}")]
                    for br in branches}

        row_iter = []
        y = 0
        for rt in row_splits(H, W):
            row_iter.append((y, rt))
            y += rt

        # start[br]: buffer index holding the branch input; with a shared
        # sequential pair, branch 1 starts in the slot branch 0 doesn't end in
        start = {}
        for bi, br in enumerate(branches):
            start[br] = (bi * (nlayers % 2 + 1)) % 2 if seq else 0

        def load_x(br):
            nchunk = 8 if W == 128 else 4
            rows = [Hp // nchunk] * nchunk
            rows[-1] += Hp - sum(rows)
            r0 = 0
            for rws in rows:
                for gg in range(2):
                    eng = nc.sync if gg == 0 else nc.scalar
                    eng.dma_start(
                        bufs[br][start[br]][:, gg, r0 * Wp:(r0 + rws) * Wp],
                        x[gg, :, r0:r0 + rws, :].rearrange("p h w -> p (h w)"))
                r0 += rws
            if seq and br != branches[0]:
                return  # shared pair: borders already zeroed by first branch
            b1 = bufs[br][1 - start[br]]
            nc.vector.memset((b1.bitcast(F32) if DT_MM == F32R else b1)[:], 0.0)

        def run_layer(br, layer):
            wr = _load_wmid(nc, wp, wmid, br, layer)
            src, dst = bufs[br][cur[br]], bufs[br][1 - cur[br]]

            def dst_write(coh, lr0, rt, pv, dst=dst, layer=layer, br=br):
                ov = dst[:, coh, :L].rearrange("p (h w) -> p h w", w=Wp)[
                    :, lr0 + 1:lr0 + 1 + rt, 1:W + 1]
                epi(ov, pv, bia[:, (br * 4 + layer) * 2 + coh])

            conv_tiles_from(nc, lambda g, src=src: src[:, g], dst_write, wr,
                            H, W, row_iter)
            cur[br] = 1 - cur[br]

        def run_final(br):
            _, bfin, NOUT = finals[br]
            cout = cout_c if br == 0 else cout_b
            _final_from(tc, nc, wrts[br], stp,
                        lambda g, br=br: bufs[br][cur[br]][:, g, :L],
                        bfin, cout, NOUT, H, W, list(range(H)), ps_fa, ps_fb)

        cur = dict(start)
        if seq:
            for br in branches:
                load_x(br)
                for layer in range(nlayers):
                    run_layer(br, layer)
                if do_final:
                    run_final(br)
        else:
            for br in branches:
                load_x(br)
            for layer in range(nlayers):
                for br in branches:
                    run_layer(br, layer)
            if do_final:
                for br in branches:
                    run_final(br)


def _level0_stream(tc, nc, x0, acts, wmid, bia, finals, cout_c, cout_b,
                   branches, H, W, nlayers, do_final, conv_tiles_from, epi,
                   ps_fa, ps_fb, sp, op, wp, wrts, stp):
    Hp, Wp = H + 2, W + 2
    SMAX = max(STRIPS0)
    strips = []
    y = 0
    for S in STRIPS0:
        strips.append((y, S))
        y += S
    assert y == H

    if True:
        srcs_br = {br: [x0] + [acts[2 * bi + (i % 2)] for i in range(nlayers)]
                   for bi, br in enumerate(branches)}
        for layer in range(nlayers):
            for br in branches:
                srcs = srcs_br[br]
                wr = _load_wmid(nc, wp, wmid, br, layer)
                src_d, dst_d = srcs[layer], srcs[layer + 1]
                for (y0, S) in strips:
                    xst = sp.tile([128, 2, (SMAX + 2) * Wp + 2], DT_MM,
                                  name=f"xs{br}{layer}{y0}", tag="xst")
                    for gg in range(2):
                        eng = nc.sync if gg == 0 else nc.scalar
                        eng.dma_start(
                            xst[:, gg, :(S + 2) * Wp],
                            src_d[gg, :, y0:y0 + S + 2, :].rearrange("p h w -> p (h w)"))
                    ost = op.tile([128, 2, SMAX, Wp], DT_MM,
                                  name=f"os{br}{layer}{y0}", tag="ost")
                    # zero the junk columns so full-width rows can be DMA'd
                    # out in one contiguous chunk per channel (they land on
                    # the zero borders of the DRAM buffer, which is correct)
                    nc.vector.memset((ost.bitcast(F32) if DT_MM == F32R else ost)[:, :, :S, 0:1], 0.0)
                    nc.vector.memset((ost.bitcast(F32) if DT_MM == F32R else ost)[:, :, :S, Wp - 1:Wp], 0.0)

                    def dst_write(coh, lr0, rt, pv, ost=ost, layer=layer):
                        epi(ost[:, coh, lr0:lr0 + rt, 1:W + 1], pv,
                            bia[:, (br * 4 + layer) * 2 + coh])

                    row_iter = []
                    ly = 0
                    for rt in strip_row_splits(S):
                        row_iter.append((ly, rt))
                        ly += rt
                    conv_tiles_from(nc, lambda g, xst=xst: xst[:, g], dst_write,
                                    wr, H, W, row_iter)
                    for gg in range(2):
                        eng = nc.scalar if gg == 0 else nc.gpsimd
                        eng.dma_start(
                            dst_d[gg, :, y0 + 1:y0 + 1 + S, :],
                            ost[:, gg, :S, :])

        if do_final:
            for br in branches:
                _, bfin, NOUT = finals[br]
                cout = cout_c if br == 0 else cout_b
                src_d = srcs_br[br][nlayers]
                for (y0, S) in strips:
                    xst = sp.tile([128, 2, (SMAX + 2) * Wp + 2], DT_MM,
                                  name=f"fx{br}{y0}", tag="xst")
                    for gg in range(2):
                        eng = nc.sync if gg == 0 else nc.scalar
                        eng.dma_start(
                            xst[:, gg, :(S + 2) * Wp],
                            src_d[gg, :, y0:y0 + S + 2, :].rearrange("p h w -> p (h w)"))

                    def src_flat(g, xst=xst):
                        return xst[:, g]
                    _final_from(tc, nc, wrts[br], stp, src_flat, bfin,
                                cout, NOUT, H, W, list(range(y0, y0 + S)),
                                ps_fa, ps_fb, local_y0=y0)


_finctr = [0]


def _final_from(tc, nc, wrt, stp, src_flat, bfin, cout, NOUT, H, W, rows,
                ps_fa, ps_fb, local_y0=0):
    """Form-A final conv over the given output rows. src_flat(g): flat padded
    AP whose row 0 == padded row local_y0."""
    Wp = W + 2
    M = W
    if NOUT == 720:
        nslices = [(0, 384), (384, 336)]
    else:
        nslices = [(0, NOUT)]
    _finctr[0] += 1
    for i, y in enumerate(rows):
        ly = y - local_y0
        stag = stp.tile([128, NOUT], F32, name=f"st{_finctr[0]}_{i}", tag=f"st{NOUT}")
        for (off, nsl) in nslices:
            tag = ("fa" + str(off)) if NOUT == 720 else "fb"
            pool = ps_fa if NOUT == 720 else ps_fb
            p = pool.tile([128, 512], F32, name=f"pf{_finctr[0]}_{i}_{off}", tag=tag)
            k = 0
            for t in range(9):
                dy, dx = t // 3, t % 3
                for g in range(2):
                    lhs = src_flat(g)[:, (ly + dy) * Wp + dx:(ly + dy) * Wp + dx + W]
                    nc.tensor.matmul(
                        p[:M, :nsl], lhs, wrt[:, t * 2 + g, off:off + nsl],
                        start=(k == 0), stop=(k == 17))
                    k += 1
            nc.vector.tensor_tensor(stag[:M, off:off + nsl], p[:M, :nsl],
                                    bfin[:M, off:off + nsl], ADD)
        nc.sync.dma_start(cout[y * W:y * W + M, :], stag[:M, :])


# ---------------- host side ----------------

def round_tf32(a: np.ndarray) -> np.ndarray:
    b = np.ascontiguousarray(a, dtype=np.float32).view(np.uint32)
    r = (b + 0x1000) & np.uint32(0xFFFFE000)
    return r.view(np.float32)


def to_mm(a: np.ndarray) -> np.ndarray:
    if USE_BF16:
        return np.asarray(a, np.float32).astype(ml_dtypes.bfloat16)
    return round_tf32(a)


_CACHE = {}


def _get_program():
    if "nc" not in _CACHE:
        _CACHE["nc"] = build_program()
    return _CACHE["nc"]


def _prep_shared(cls_w, cls_b, bbox_w, bbox_b, cls_out_w, cls_out_b,
                 bbox_out_w, bbox_out_b):
    wmid = np.empty((2, 4, 9, 2, 128, 256), np.float32)
    for b, wsrc in ((0, cls_w), (1, bbox_w)):
        for l in range(4):
            for t in range(9):
                dy, dx = t // 3, t % 3
                for g in range(2):
                    wmid[b, l, t, g] = wsrc[l, dy, dx, g * 128:(g + 1) * 128, :]
    wmid = to_mm(wmid.reshape(144, 128, 256))

    bmid = np.empty((2, 4, 2, 128, 1), np.float32)
    for b, bsrc in ((0, cls_b), (1, bbox_b)):
        for l in range(4):
            for coh in range(2):
                bmid[b, l, coh, :, 0] = bsrc[l, coh * 128:(coh + 1) * 128]
    bmid = bmid.reshape(16, 128, 1)

    wcls = np.empty((9, 2, 128, NCLS), np.float32)
    wbox = np.empty((9, 2, 128, NBOX), np.float32)
    for t in range(9):
        dy, dx = t // 3, t % 3
        for g in range(2):
            wcls[t, g] = cls_out_w[dy, dx, g * 128:(g + 1) * 128, :]
            wbox[t, g] = bbox_out_w[dy, dx, g * 128:(g + 1) * 128, :]
    wcls = to_mm(wcls.reshape(18, 128, NCLS))
    wbox = to_mm(wbox.reshape(18, 128, NBOX))

    bclsr = np.broadcast_to(cls_out_b.astype(np.float32), (128, NCLS)).copy()
    bboxr = np.broadcast_to(bbox_out_b.astype(np.float32), (128, NBOX)).copy()
    return dict(wmid=wmid, bmid=bmid, wcls=wcls, bcls=bclsr, wbox=wbox, bbox=bboxr)


def _prep_x(x):
    H, W, _ = x.shape
    dt = ml_dtypes.bfloat16 if USE_BF16 else np.float32
    out = np.zeros((2, 128, H + 2, W + 2), dt)
    xt = to_mm(x).transpose(2, 0, 1).reshape(2, 128, H, W)
    out[:, :, 1:H + 1, 1:W + 1] = xt
    return out


def kernel(x0, x1, x2, cls_w, cls_b, bbox_w, bbox_b,
           cls_out_w, cls_out_b, bbox_out_w, bbox_out_b):
    nc = _get_program()
    shared = _prep_shared(cls_w, cls_b, bbox_w, bbox_b, cls_out_w, cls_out_b,
                          bbox_out_w, bbox_out_b)
    in_maps = []
    for n in range(8):
        m = dict(shared)
        m["x0"] = _prep_x(np.asarray(x0)[n])
        m["x1"] = _prep_x(np.asarray(x1)[n])
        m["x2"] = _prep_x(np.asarray(x2)[n])
        in_maps.append(m)
    res = run_bass_kernel_spmd(nc, in_maps, core_ids=list(range(8)),
                               trace=_CACHE.get("trace", False))
    _CACHE["exec_time_ns"] = res.exec_time_ns
    outs = []
    for name, (H, W), NC_ in (("c0", LEVELS[0], NCLS), ("c1", LEVELS[1], NCLS),
                              ("c2", LEVELS[2], NCLS), ("b0", LEVELS[0], NBOX),
                              ("b1", LEVELS[1], NBOX), ("b2", LEVELS[2], NBOX)):
        outs.append(np.stack([res.results[n][name].reshape(H, W, NC_)
                              for n in range(8)]))
    return tuple(outs)


# revision 34
# speedup vs baseline: 1.0007x; 1.0007x over previous
"""AnchorHead (RetinaNet-style) Trainium2 kernel.

8 NeuronCores, data-parallel over batch (1 image per core), weights
replicated, no collectives. Per core: for each pyramid level (128^2, 64^2,
32^2) and each branch (cls, bbox): 4x [3x3 conv 256->256 + ReLU] then a
final 3x3 conv (720 or 36 channels).

Conv-as-matmul with fp32 PSUM accumulation; operand dtype is bf16
(USE_BF16, rel_err ~5.6e-3) or float32r/TF32 (~5.8e-4), both host-rounded
so DMA needs no on-chip conversion:
 - intermediate convs ("form B"): activations [ci(128 part) x 2, padded
   spatial] as the moving operand; weights [ci,co] stationary; PSUM
   [co, pixels]; 18 accumulating matmuls per tile (9 taps x 2 ci halves);
   row-aligned pixel tiles (N = rows*(W+2)) so a fused bias+ReLU DVE
   epilogue writes only interior columns of the next padded buffer.
 - final convs ("form A"): activations stationary [ci, row of W pixels],
   weights [ci, co] moving; PSUM [pix, co]; outputs DMA out contiguously
   in NHWC.

Level 0 streams row-strips through per-branch DRAM ping-pongs (full-width
strip writes land their zeroed junk columns on the zero borders); levels
1/2 are SBUF-resident. All tile pools are opened concurrently on disjoint
SBUF so the Tile scheduler overlaps branches, levels, and final phases;
weights (intermediate + final) are preloaded on separate DMA rings.
"""
import numpy as np

import concourse.bass as bass
import concourse.tile as tile
from concourse import bacc, mybir
from concourse.bass_utils import run_bass_kernel_spmd

import ml_dtypes

F32 = mybir.dt.float32
F32R = mybir.dt.float32r
BF16 = mybir.dt.bfloat16
USE_BF16 = True
DT_MM = BF16 if USE_BF16 else F32R   # matmul-operand storage dtype
ADD = mybir.AluOpType.add
MAX = mybir.AluOpType.max

LEVELS = [(128, 128), (64, 64), (32, 32)]
C = 256
NCLS = 720
NBOX = 36
STRIPS0 = [12] + [24] * 4 + [20]   # L0 strip heights (sum 128); small lead-in strip lets PE start early


def row_splits(H, W):
    # rows-per-matmul-tile for resident levels; every N = rows*(W+2) >= 256
    if H == 128:
        return [3] * 42 + [2]
    if H == 64:
        return [6] * 10 + [4]
    if H == 32:
        return [11, 11, 10]
    raise ValueError(H)


def strip_row_splits(S):
    sp = [3] * (S // 3)
    if S % 3:
        sp.append(S % 3)  # 2-row tail tile (N=260) for S=20
    return sp


def build_program(levels=(0, 1, 2), branches=(0, 1), nlayers=4, do_final=True):
    nc = bacc.Bacc("TRN2", target_bir_lowering=False, debug=False, num_devices=8)

    xs = []
    for li, (H, W) in enumerate(LEVELS):
        xs.append(nc.declare_dram_parameter(
            f"x{li}", [2, 128, H + 2, W + 2], DT_MM, isOutput=False))
    wmid = nc.declare_dram_parameter("wmid", [144, 128, 256], DT_MM, isOutput=False)
    bmid = nc.declare_dram_parameter("bmid", [16, 128, 1], F32, isOutput=False)
    wcls = nc.declare_dram_parameter("wcls", [18, 128, NCLS], DT_MM, isOutput=False)
    bcls = nc.declare_dram_parameter("bcls", [128, NCLS], F32, isOutput=False)
    wbox = nc.declare_dram_parameter("wbox", [18, 128, NBOX], DT_MM, isOutput=False)
    bbox = nc.declare_dram_parameter("bbox", [128, NBOX], F32, isOutput=False)
    couts, bouts = [], []
    for li, (H, W) in enumerate(LEVELS):
        couts.append(nc.declare_dram_parameter(f"c{li}", [H * W, NCLS], F32, isOutput=True))
        bouts.append(nc.declare_dram_parameter(f"b{li}", [H * W, NBOX], F32, isOutput=True))

    H0, W0 = LEVELS[0]
    acts = [nc.dram_tensor(f"act{i}", [2, 128, H0 + 2, W0 + 2], DT_MM) for i in range(4)]

    with tile.TileContext(nc) as tc:
        _build_tile(tc, nc, xs, wmid, bmid, wcls, bcls, wbox, bbox, couts, bouts,
                    acts, levels, branches, nlayers, do_final)
    nc.compile()
    return nc


def _build_tile(tc, nc, xs, wmid, bmid, wcls, bcls, wbox, bbox, couts, bouts,
                acts, levels, branches, nlayers, do_final):
    from contextlib import ExitStack

    with ExitStack() as g:
        glob = g.enter_context(tc.tile_pool(name="glob", bufs=1))
        ps_mm = g.enter_context(tc.tile_pool(name="ps_mm", bufs=5, space="PSUM"))
        ps_fa = g.enter_context(tc.tile_pool(name="ps_fa", bufs=1, space="PSUM"))
        ps_fb = g.enter_context(tc.tile_pool(name="ps_fb", bufs=1, space="PSUM"))

        bia = glob.tile([128, 16, 1], F32)
        nc.sync.dma_start(bia[:], bmid.rearrange("k p o -> p k o"))
        bclst = glob.tile([128, NCLS], F32)
        nc.sync.dma_start(bclst[:], bcls[:, :])
        bboxt = glob.tile([128, NBOX], F32)
        nc.sync.dma_start(bboxt[:], bbox[:, :])

        if 0 in levels:
            Hp, Wp = LEVELS[0][0] + 2, LEVELS[0][1] + 2
            zt = glob.tile([128, 2, Wp], DT_MM)
            nc.vector.memset((zt.bitcast(F32) if DT_MM == F32R else zt)[:], 0.0)
            # only the top/bottom padded rows need zeroing here: interior rows
            # (including their border columns) are fully written by every
            # layer's full-width strip DMAs
            for a in acts:
                nc.sync.dma_start(a[:, :, 0, :].rearrange("g p w -> p g w"), zt[:])
                nc.scalar.dma_start(a[:, :, Hp - 1, :].rearrange("g p w -> p g w"), zt[:])

        mmctr = [0]

        def conv_tiles_from(nc, src_flat, dst_write, wr, H, W, row_iter):
            Wp = W + 2
            for (lr0, rt) in row_iter:
                for coh in range(2):
                    Npix = rt * Wp
                    p = ps_mm.tile([128, 512], F32, name=f"mm{mmctr[0]}", tag="mm")
                    mmctr[0] += 1
                    k = 0
                    for t in range(9):
                        dy, dx = t // 3, t % 3
                        off = (dy - 1) * Wp + (dx - 1)
                        for gg in range(2):
                            b0 = (lr0 + 1) * Wp + 1 + off
                            nc.tensor.matmul(
                                p[:, :Npix],
                                wr[:, t * 2 + gg, coh * 128:(coh + 1) * 128],
                                src_flat(gg)[:, b0:b0 + Npix],
                                start=(k == 0), stop=(k == 17))
                            k += 1
                    pv = p[:, :Npix].rearrange("p (r w) -> p r w", w=Wp)[:, :, :W]
                    dst_write(coh, lr0, rt, pv)

        def epi(out_ap, pv, bias_ap):
            nc.vector.tensor_scalar(out_ap, pv, bias_ap, 0.0, ADD, MAX)

        finals = {}
        for br in branches:
            finals[br] = ((wcls, bclst, NCLS) if br == 0 else (wbox, bboxt, NBOX))

        # global pools: intermediate weights, final weights (loaded once),
        # output staging. Level pools are opened concurrently (disjoint SBUF)
        # so the scheduler can overlap phases across levels freely.
        wpg = g.enter_context(tc.tile_pool(name="wpg", bufs=2))
        stp = g.enter_context(tc.tile_pool(name="stp", bufs=2))
        wrts = {}
        if do_final:
            for br in branches:
                wfin, _, NOUT = finals[br]
                wfp = g.enter_context(tc.tile_pool(name=f"fwg{NOUT}", bufs=1))
                wrt = wfp.tile([128, 18, NOUT], DT_MM, name=f"fwg{NOUT}t")
                nc.gpsimd.dma_start(wrt[:], wfin[:, :, :].rearrange("k p m -> p k m"))
                wrts[br] = wrt
        pools = {}
        if 0 in levels:
            pools["sp0"] = g.enter_context(tc.tile_pool(name="sp0", bufs=2))
            pools["op0"] = g.enter_context(tc.tile_pool(name="op0", bufs=2))
        for li in levels:
            if li != 0:
                pools[f"ab{li}"] = g.enter_context(
                    tc.tile_pool(name=f"ab{LEVELS[li][1]}", bufs=1))

        for li in levels:
            H, W = LEVELS[li]
            if li == 0:
                _level0_stream(tc, nc, xs[0], acts, wmid, bia, finals,
                               couts[0], bouts[0], branches, H, W, nlayers,
                               do_final, conv_tiles_from, epi, ps_fa, ps_fb,
                               pools["sp0"], pools["op0"], wpg, wrts, stp)
            else:
                _level_resident(tc, nc, xs[li], wmid, bia, finals,
                                couts[li], bouts[li], branches, H, W, nlayers,
                                do_final, conv_tiles_from, epi, ps_fa, ps_fb,
                                pools[f"ab{li}"], wpg, wrts, stp,
                                seq=(li == 2))


def _load_wmid(nc, wp, wmid, br, layer):
    wr = wp.tile([128, 18, 256], DT_MM, name=f"w{br}{layer}_{_finctr[0]}", tag="wmid")
    i0 = (br * 4 + layer) * 18
    nc.gpsimd.dma_start(wr[:], wmid[i0:i0 + 18].rearrange("k p m -> p k m"))
    return wr


def _level_resident(tc, nc, x, wmid, bia, finals, cout_c, cout_b, branches,
                    H, W, nlayers, do_final, conv_tiles_from, epi, ps_fa, ps_fb,
                    ab, wp, wrts, stp, seq=False):
    """Fully SBUF-resident level.

    seq=False: each branch gets its own buffer pair; layers interleaved
    across branches so the scheduler always has an independent stream.
    seq=True (level 0, buffers too big for two pairs): one shared pair,
    branches processed sequentially."""
    Hp, Wp = H + 2, W + 2
    L = Hp * Wp
    if True:
        if seq:
            pair = [ab.tile([128, 2, L + 2], DT_MM, name=f"bA{W}"),
                    ab.tile([128, 2, L + 2], DT_MM, name=f"bB{W}")]
            bufs = {br: pair for br in branches}
        else:
            bufs = {br: [ab.tile([128, 2, L + 2], DT_MM, name=f"bA{br}{W}"),
                         ab.tile([128, 2, L + 2], DT_MM, name=f"bB{br}{W}")]
                    for br in branches}

        row_iter = []
        y = 0
        for rt in row_splits(H, W):
            row_iter.append((y, rt))
            y += rt

        # start[br]: buffer index holding the branch input; with a shared
        # sequential pair, branch 1 starts in the slot branch 0 doesn't end in
        start = {}
        for bi, br in enumerate(branches):
            start[br] = (bi * (nlayers % 2 + 1)) % 2 if seq else 0

        def load_x(br):
            nchunk = 8 if W == 128 else 4
            rows = [Hp // nchunk] * nchunk
            rows[-1] += Hp - sum(rows)
            r0 = 0
            for rws in rows:
                for gg in range(2):
                    eng = nc.sync if gg == 0 else nc.scalar
                    eng.dma_start(
                        bufs[br][start[br]][:, gg, r0 * Wp:(r0 + rws) * Wp],
                        x[gg, :, r0:r0 + rws, :].rearrange("p h w -> p (h w)"))
                r0 += rws
            if seq and br != branches[0]:
                return  # shared pair: borders already zeroed by first branch
            b1 = bufs[br][1 - start[br]]
            nc.vector.memset((b1.bitcast(F32) if DT_MM == F32R else b1)[:], 0.0)

        def run_layer(br, layer):
            wr = _load_wmid(nc, wp, wmid, br, layer)
            src, dst = bufs[br][cur[br]], bufs[br][1 - cur[br]]

            def dst_write(coh, lr0, rt, pv, dst=dst, layer=layer, br=br):
                ov = dst[:, coh, :L].rearrange("p (h w) -> p h w", w=Wp)[
                    :, lr0 + 1:lr0 + 1 + rt, 1:W + 1]
                epi(ov, pv, bia[:, (br * 4 + layer) * 2 + coh])

            conv_tiles_from(nc, lambda g, src=src: src[:, g], dst_write, wr,
                            H, W, row_iter)
            cur[br] = 1 - cur[br]

        def run_final(br):
            _, bfin, NOUT = finals[br]
            cout = cout_c if br == 0 else cout_b
            _final_from(tc, nc, wrts[br], stp,
                        lambda g, br=br: bufs[br][cur[br]][:, g, :L],
                        bfin, cout, NOUT, H, W, list(range(H)), ps_fa, ps_fb)

        cur = dict(start)
        if seq:
            for br in branches:
                load_x(br)
                for layer in range(nlayers):
                    run_layer(br, layer)
                if do_final:
                    run_final(br)
        else:
            for br in branches:
                load_x(br)
            for layer in range(nlayers):
                for br in branches:
                    run_layer(br, layer)
            if do_final:
                for br in branches:
                    run_final(br)


def _level0_stream(tc, nc, x0, acts, wmid, bia, finals, cout_c, cout_b,
                   branches, H, W, nlayers, do_final, conv_tiles_from, epi,
                   ps_fa, ps_fb, sp, op, wp, wrts, stp):
    Hp, Wp = H + 2, W + 2
    SMAX = max(STRIPS0)
    strips = []
    y = 0
    for S in STRIPS0:
        strips.append((y, S))
        y += S
    assert y == H

    if True:
        srcs_br = {br: [x0] + [acts[2 * bi + (i % 2)] for i in range(nlayers)]
                   for bi, br in enumerate(branches)}
        for layer in range(nlayers):
            for br in branches:
                srcs = srcs_br[br]
                wr = _load_wmid(nc, wp, wmid, br, layer)
                src_d, dst_d = srcs[layer], srcs[layer + 1]
                for (y0, S) in strips:
                    xst = sp.tile([128, 2, (SMAX + 2) * Wp + 2], DT_MM,
                                  name=f"xs{br}{layer}{y0}", tag="xst")
                    for gg in range(2):
                        eng = nc.sync if gg == 0 else nc.scalar
                        eng.dma_start(
                            xst[:, gg, :(S + 2) * Wp],
                            src_d[gg, :, y0:y0 + S + 2, :].rearrange("p h w -> p (h w)"))
                    ost = op.tile([128, 2, SMAX, Wp], DT_MM,
                                  name=f"os{br}{layer}{y0}", tag="ost")
                    # zero the junk columns so full-width rows can be DMA'd
                    # out in one contiguous chunk per channel (they land on
                    # the zero borders of the DRAM buffer, which is correct)
                    nc.vector.memset((ost.bitcast(F32) if DT_MM == F32R else ost)[:, :, :S, 0:1], 0.0)
                    nc.vector.memset((ost.bitcast(F32) if DT_MM == F32R else ost)[:, :, :S, Wp - 1:Wp], 0.0)

                    def dst_write(coh, lr0, rt, pv, ost=ost, layer=layer):
                        epi(ost[:, coh, lr0:lr0 + rt, 1:W + 1], pv,
                            bia[:, (br * 4 + layer) * 2 + coh])

                    row_iter = []
                    ly = 0
                    for rt in strip_row_splits(S):
                        row_iter.append((ly, rt))
                        ly += rt
                    conv_tiles_from(nc, lambda g, xst=xst: xst[:, g], dst_write,
                                    wr, H, W, row_iter)
                    for gg in range(2):
                        eng = nc.scalar if gg == 0 else nc.gpsimd
                        eng.dma_start(
                            dst_d[gg, :, y0 + 1:y0 + 1 + S, :],
                            ost[:, gg, :S, :])

        if do_final:
            for br in branches:
                _, bfin, NOUT = finals[br]
                cout = cout_c if br == 0 else cout_b
                src_d = srcs_br[br][nlayers]
                for (y0, S) in strips:
                    xst = sp.tile([128, 2, (SMAX + 2) * Wp + 2], DT_MM,
                                  name=f"fx{br}{y0}", tag="xst")
                    for gg in range(2):
                        eng = nc.sync if gg == 0 else nc.scalar
                        eng.dma_start(
                            xst[:, gg, :(S + 2) * Wp],
                            src_d[gg, :, y0:y0 + S + 2, :].rearrange("p h w -> p (h w)"))

                    def src_flat(g, xst=xst):
                        return xst[:, g]
                    _final_from(tc, nc, wrts[br], stp, src_flat, bfin,
                                cout, NOUT, H, W, list(range(y0, y0 + S)),
                                ps_fa, ps_fb, local_y0=y0)


_finctr = [0]


def _final_from(tc, nc, wrt, stp, src_flat, bfin, cout, NOUT, H, W, rows,
                ps_fa, ps_fb, local_y0=0):
    """Form-A final conv over the given output rows. src_flat(g): flat padded
    AP whose row 0 == padded row local_y0."""
    Wp = W + 2
    M = W
    if NOUT == 720:
        nslices = [(0, 384), (384, 336)]
    else:
        nslices = [(0, NOUT)]
    _finctr[0] += 1
    for i, y in enumerate(rows):
        ly = y - local_y0
        stag = stp.tile([128, NOUT], F32, name=f"st{_finctr[0]}_{i}", tag=f"st{NOUT}")
        for (off, nsl) in nslices:
            tag = ("fa" + str(off)) if NOUT == 720 else "fb"
            pool = ps_fa if NOUT == 720 else ps_fb
            p = pool.tile([128, 512], F32, name=f"pf{_finctr[0]}_{i}_{off}", tag=tag)
            k = 0
            for t in range(9):
                dy, dx = t // 3, t % 3
                for g in range(2):
                    lhs = src_flat(g)[:, (ly + dy) * Wp + dx:(ly + dy) * Wp + dx + W]
                    nc.tensor.matmul(
                        p[:M, :nsl], lhs, wrt[:, t * 2 + g, off:off + nsl],
                        start=(k == 0), stop=(k == 17))
                    k += 1
            nc.vector.tensor_tensor(stag[:M, off:off + nsl], p[:M, :nsl],
                                    bfin[:M, off:off + nsl], ADD)
        nc.sync.dma_start(cout[y * W:y * W + M, :], stag[:M, :])


# ---------------- host side ----------------

def round_tf32(a: np.ndarray) -> np.ndarray:
    b = np.ascontiguousarray(a, dtype=np.float32).view(np.uint32)
    r = (b + 0x1000) & np.uint32(0xFFFFE000)
    return r.view(np.float32)


def to_mm(a: np.ndarray) -> np.ndarray:
    if USE_BF16:
        return np.asarray(a, np.float32).astype(ml_dtypes.bfloat16)
    return round_tf32(a)


_CACHE = {}


def _get_program():
    if "nc" not in _CACHE:
        _CACHE["nc"] = build_program()
    return _CACHE["nc"]


def _prep_shared(cls_w, cls_b, bbox_w, bbox_b, cls_out_w, cls_out_b,
                 bbox_out_w, bbox_out_b):
    wmid = np.empty((2, 4, 9, 2, 128, 256), np.float32)
    for b, wsrc in ((0, cls_w), (1, bbox_w)):
        for l in range(4):
            for t in range(9):
                dy, dx = t // 3, t % 3
                for g in range(2):
                    wmid[b, l, t, g] = wsrc[l, dy, dx, g * 128:(g + 1) * 128, :]
    wmid = to_mm(wmid.reshape(144, 128, 256))

    bmid = np.empty((2, 4, 2, 128, 1), np.float32)
    for b, bsrc in ((0, cls_b), (1, bbox_b)):
        for l in range(4):
            for coh in range(2):
                bmid[b, l, coh, :, 0] = bsrc[l, coh * 128:(coh + 1) * 128]
    bmid = bmid.reshape(16, 128, 1)

    wcls = np.empty((9, 2, 128, NCLS), np.float32)
    wbox = np.empty((9, 2, 128, NBOX), np.float32)
    for t in range(9):
        dy, dx = t // 3, t % 3
        for g in range(2):
            wcls[t, g] = cls_out_w[dy, dx, g * 128:(g + 1) * 128, :]
            wbox[t, g] = bbox_out_w[dy, dx, g * 128:(g + 1) * 128, :]
    wcls = to_mm(wcls.reshape(18, 128, NCLS))
    wbox = to_mm(wbox.reshape(18, 128, NBOX))

    bclsr = np.broadcast_to(cls_out_b.astype(np.float32), (128, NCLS)).copy()
    bboxr = np.broadcast_to(bbox_out_b.astype(np.float32), (128, NBOX)).copy()
    return dict(wmid=wmid, bmid=bmid, wcls=wcls, bcls=bclsr, wbox=wbox, bbox=bboxr)


def _prep_x(x):
    H, W, _ = x.shape
    dt = ml_dtypes.bfloat16 if USE_BF16 else np.float32
    out = np.zeros((2, 128, H + 2, W + 2), dt)
    xt = to_mm(x).transpose(2, 0, 1).reshape(2, 128, H, W)
    out[:, :, 1:H + 1, 1:W + 1] = xt
    return out


def kernel(x0, x1, x2, cls_w, cls_b, bbox_w, bbox_b,
           cls_out_w, cls_out_b, bbox_out_w, bbox_out_b):
    nc = _get_program()
    cls_w, cls_b, bbox_w, bbox_b = (np.asarray(a, np.float32) for a in
                                    (cls_w, cls_b, bbox_w, bbox_b))
    cls_out_w, cls_out_b, bbox_out_w, bbox_out_b = (
        np.asarray(a, np.float32) for a in
        (cls_out_w, cls_out_b, bbox_out_w, bbox_out_b))
    shared = _prep_shared(cls_w, cls_b, bbox_w, bbox_b, cls_out_w, cls_out_b,
                          bbox_out_w, bbox_out_b)
    in_maps = []
    for n in range(8):
        m = dict(shared)
        m["x0"] = _prep_x(np.asarray(x0)[n])
        m["x1"] = _prep_x(np.asarray(x1)[n])
        m["x2"] = _prep_x(np.asarray(x2)[n])
        in_maps.append(m)
    res = run_bass_kernel_spmd(nc, in_maps, core_ids=list(range(8)),
                               trace=_CACHE.get("trace", False))
    _CACHE["exec_time_ns"] = res.exec_time_ns
    outs = []
    for name, (H, W), NC_ in (("c0", LEVELS[0], NCLS), ("c1", LEVELS[1], NCLS),
                              ("c2", LEVELS[2], NCLS), ("b0", LEVELS[0], NBOX),
                              ("b1", LEVELS[1], NBOX), ("b2", LEVELS[2], NBOX)):
        outs.append(np.stack([res.results[n][name].reshape(H, W, NC_)
                              for n in range(8)]))
    return tuple(outs)


# revision 35
# speedup vs baseline: 1.2617x; 1.2608x over previous
"""AnchorHead (RetinaNet-style) Trainium2 kernel.

8 NeuronCores, data-parallel over batch (1 image per core), weights
replicated, no collectives. Per core: for each pyramid level (128^2, 64^2,
32^2) and each branch (cls, bbox): 4x [3x3 conv 256->256 + ReLU] then a
final 3x3 conv (720 or 36 channels).

Conv-as-matmul with fp32 PSUM accumulation; operand dtype is bf16
(USE_BF16, rel_err ~5.6e-3) or float32r/TF32 (~5.8e-4), both host-rounded
so DMA needs no on-chip conversion:
 - intermediate convs ("form B"): activations [ci(128 part) x 2, padded
   spatial] as the moving operand; weights [ci,co] stationary; PSUM
   [co, pixels]; 18 accumulating matmuls per tile (9 taps x 2 ci halves);
   row-aligned pixel tiles (N = rows*(W+2)) so a fused bias+ReLU DVE
   epilogue writes only interior columns of the next padded buffer.
 - final convs ("form A"): activations stationary [ci, row of W pixels],
   weights [ci, co] moving; PSUM [pix, co]; outputs DMA out contiguously
   in NHWC.

Level 0 streams row-strips through per-branch DRAM ping-pongs (full-width
strip writes land their zeroed junk columns on the zero borders); levels
1/2 are SBUF-resident. All tile pools are opened concurrently on disjoint
SBUF so the Tile scheduler overlaps branches, levels, and final phases;
weights (intermediate + final) are preloaded on separate DMA rings.
"""
import numpy as np

import concourse.bass as bass
import concourse.tile as tile
from concourse import bacc, mybir
from concourse.bass_utils import run_bass_kernel_spmd

import ml_dtypes

F32 = mybir.dt.float32
F32R = mybir.dt.float32r
BF16 = mybir.dt.bfloat16
USE_BF16 = True
DT_MM = BF16 if USE_BF16 else F32R   # matmul-operand storage dtype
ADD = mybir.AluOpType.add
MAX = mybir.AluOpType.max

LEVELS = [(128, 128), (64, 64), (32, 32)]
C = 256
NCLS = 720
NBOX = 36
STRIPS0 = [12] + [24] * 4 + [20]   # L0 strip heights (sum 128); small lead-in strip lets PE start early


def row_splits(H, W):
    # rows-per-matmul-tile for resident levels; every N = rows*(W+2) >= 256
    if H == 128:
        return [3] * 42 + [2]
    if H == 64:
        return [6] * 10 + [4]
    if H == 32:
        return [11, 11, 10]
    raise ValueError(H)


def strip_row_splits(S):
    sp = [3] * (S // 3)
    if S % 3:
        sp.append(S % 3)  # 2-row tail tile (N=260) for S=20
    return sp


def build_program(levels=(0, 1, 2), branches=(0, 1), nlayers=4, do_final=True):
    nc = bacc.Bacc("TRN2", target_bir_lowering=False, debug=False, num_devices=8)

    xs = []
    for li, (H, W) in enumerate(LEVELS):
        xs.append(nc.declare_dram_parameter(
            f"x{li}", [2, 128, H + 2, W + 2], DT_MM, isOutput=False))
    wmid = nc.declare_dram_parameter("wmid", [144, 128, 256], DT_MM, isOutput=False)
    bmid = nc.declare_dram_parameter("bmid", [16, 128, 1], F32, isOutput=False)
    wcls = nc.declare_dram_parameter("wcls", [18, 128, NCLS], DT_MM, isOutput=False)
    bcls = nc.declare_dram_parameter("bcls", [128, NCLS], F32, isOutput=False)
    wbox = nc.declare_dram_parameter("wbox", [18, 128, NBOX], DT_MM, isOutput=False)
    bbox = nc.declare_dram_parameter("bbox", [128, NBOX], F32, isOutput=False)
    couts, bouts = [], []
    for li, (H, W) in enumerate(LEVELS):
        couts.append(nc.declare_dram_parameter(f"c{li}", [H * W, NCLS], F32, isOutput=True))
        bouts.append(nc.declare_dram_parameter(f"b{li}", [H * W, NBOX], F32, isOutput=True))

    H0, W0 = LEVELS[0]
    acts = [nc.dram_tensor(f"act{i}", [2, 128, H0 + 2, W0 + 2], DT_MM) for i in range(4)]

    with tile.TileContext(nc) as tc:
        _build_tile(tc, nc, xs, wmid, bmid, wcls, bcls, wbox, bbox, couts, bouts,
                    acts, levels, branches, nlayers, do_final)
    nc.compile()
    return nc


def _build_tile(tc, nc, xs, wmid, bmid, wcls, bcls, wbox, bbox, couts, bouts,
                acts, levels, branches, nlayers, do_final):
    from contextlib import ExitStack

    with ExitStack() as g:
        glob = g.enter_context(tc.tile_pool(name="glob", bufs=1))
        ps_mm = g.enter_context(tc.tile_pool(name="ps_mm", bufs=5, space="PSUM"))
        ps_fa = g.enter_context(tc.tile_pool(name="ps_fa", bufs=1, space="PSUM"))
        ps_fb = g.enter_context(tc.tile_pool(name="ps_fb", bufs=1, space="PSUM"))

        bia = glob.tile([128, 16, 1], F32)
        nc.sync.dma_start(bia[:], bmid.rearrange("k p o -> p k o"))
        bclst = glob.tile([128, NCLS], F32)
        nc.sync.dma_start(bclst[:], bcls[:, :])
        bboxt = glob.tile([128, NBOX], F32)
        nc.sync.dma_start(bboxt[:], bbox[:, :])

        if 0 in levels:
            Hp, Wp = LEVELS[0][0] + 2, LEVELS[0][1] + 2
            zt = glob.tile([128, 2, Wp], DT_MM)
            nc.vector.memset((zt.bitcast(F32) if DT_MM == F32R else zt)[:], 0.0)
            # only the top/bottom padded rows need zeroing here: interior rows
            # (including their border columns) are fully written by every
            # layer's full-width strip DMAs
            for a in acts:
                nc.sync.dma_start(a[:, :, 0, :].rearrange("g p w -> p g w"), zt[:])
                nc.scalar.dma_start(a[:, :, Hp - 1, :].rearrange("g p w -> p g w"), zt[:])

        mmctr = [0]

        def conv_tiles_from(nc, src_flat, dst_write, wr, H, W, row_iter):
            Wp = W + 2
            for (lr0, rt) in row_iter:
                for coh in range(2):
                    Npix = rt * Wp
                    p = ps_mm.tile([128, 512], F32, name=f"mm{mmctr[0]}", tag="mm")
                    mmctr[0] += 1
                    k = 0
                    for t in range(9):
                        dy, dx = t // 3, t % 3
                        off = (dy - 1) * Wp + (dx - 1)
                        for gg in range(2):
                            b0 = (lr0 + 1) * Wp + 1 + off
                            nc.tensor.matmul(
                                p[:, :Npix],
                                wr[:, t * 2 + gg, coh * 128:(coh + 1) * 128],
                                src_flat(gg)[:, b0:b0 + Npix],
                                start=(k == 0), stop=(k == 17))
                            k += 1
                    pv = p[:, :Npix].rearrange("p (r w) -> p r w", w=Wp)[:, :, :W]
                    dst_write(coh, lr0, rt, pv)

        def epi(out_ap, pv, bias_ap):
            nc.vector.tensor_scalar(out_ap, pv, bias_ap, 0.0, ADD, MAX)

        finals = {}
        for br in branches:
            finals[br] = ((wcls, bclst, NCLS) if br == 0 else (wbox, bboxt, NBOX))

        # global pools: intermediate weights, final weights (loaded once),
        # output staging. Level pools are opened concurrently (disjoint SBUF)
        # so the scheduler can overlap phases across levels freely.
        wpg = g.enter_context(tc.tile_pool(name="wpg", bufs=2))
        stp = g.enter_context(tc.tile_pool(name="stp", bufs=2))
        wrts = {}
        if do_final:
            for br in branches:
                wfin, _, NOUT = finals[br]
                wfp = g.enter_context(tc.tile_pool(name=f"fwg{NOUT}", bufs=1))
                wrt = wfp.tile([128, 18, NOUT], DT_MM, name=f"fwg{NOUT}t")
                nc.gpsimd.dma_start(wrt[:], wfin[:, :, :].rearrange("k p m -> p k m"))
                wrts[br] = wrt
        pools = {}
        if 0 in levels:
            pools["sp0"] = g.enter_context(tc.tile_pool(name="sp0", bufs=2))
            pools["op0"] = g.enter_context(tc.tile_pool(name="op0", bufs=2))
        for li in levels:
            if li != 0:
                pools[f"ab{li}"] = g.enter_context(
                    tc.tile_pool(name=f"ab{LEVELS[li][1]}", bufs=1))

        for li in levels:
            H, W = LEVELS[li]
            if li == 0:
                _level0_stream(tc, nc, xs[0], acts, wmid, bia, finals,
                               couts[0], bouts[0], branches, H, W, nlayers,
                               do_final, conv_tiles_from, epi, ps_fa, ps_fb,
                               pools["sp0"], pools["op0"], wpg, wrts, stp)
            else:
                _level_resident(tc, nc, xs[li], wmid, bia, finals,
                                couts[li], bouts[li], branches, H, W, nlayers,
                                do_final, conv_tiles_from, epi, ps_fa, ps_fb,
                                pools[f"ab{li}"], wpg, wrts, stp,
                                seq=(li == 2))


def _load_wmid(nc, wp, wmid, br, layer):
    wr = wp.tile([128, 18, 256], DT_MM, name=f"w{br}{layer}_{_finctr[0]}", tag="wmid")
    i0 = (br * 4 + layer) * 18
    nc.gpsimd.dma_start(wr[:], wmid[i0:i0 + 18].rearrange("k p m -> p k m"))
    return wr


def _level_resident(tc, nc, x, wmid, bia, finals, cout_c, cout_b, branches,
                    H, W, nlayers, do_final, conv_tiles_from, epi, ps_fa, ps_fb,
                    ab, wp, wrts, stp, seq=False):
    """Fully SBUF-resident level.

    seq=False: each branch gets its own buffer pair; layers interleaved
    across branches so the scheduler always has an independent stream.
    seq=True (level 0, buffers too big for two pairs): one shared pair,
    branches processed sequentially."""
    Hp, Wp = H + 2, W + 2
    L = Hp * Wp
    if True:
        if seq:
            pair = [ab.tile([128, 2, L + 2], DT_MM, name=f"bA{W}"),
                    ab.tile([128, 2, L + 2], DT_MM, name=f"bB{W}")]
            bufs = {br: pair for br in branches}
        else:
            bufs = {br: [ab.tile([128, 2, L + 2], DT_MM, name=f"bA{br}{W}"),
                         ab.tile([128, 2, L + 2], DT_MM, name=f"bB{br}{W}")]
                    for br in branches}

        row_iter = []
        y = 0
        for rt in row_splits(H, W):
            row_iter.append((y, rt))
            y += rt

        # start[br]: buffer index holding the branch input; with a shared
        # sequential pair, branch 1 starts in the slot branch 0 doesn't end in
        start = {}
        for bi, br in enumerate(branches):
            start[br] = (bi * (nlayers % 2 + 1)) % 2 if seq else 0

        def load_x(br):
            nchunk = 8 if W == 128 else 4
            rows = [Hp // nchunk] * nchunk
            rows[-1] += Hp - sum(rows)
            r0 = 0
            for rws in rows:
                for gg in range(2):
                    eng = nc.sync if gg == 0 else nc.scalar
                    eng.dma_start(
                        bufs[br][start[br]][:, gg, r0 * Wp:(r0 + rws) * Wp],
                        x[gg, :, r0:r0 + rws, :].rearrange("p h w -> p (h w)"))
                r0 += rws
            if seq and br != branches[0]:
                return  # shared pair: borders already zeroed by first branch
            b1 = bufs[br][1 - start[br]]
            nc.vector.memset((b1.bitcast(F32) if DT_MM == F32R else b1)[:], 0.0)

        def run_layer(br, layer):
            wr = _load_wmid(nc, wp, wmid, br, layer)
            src, dst = bufs[br][cur[br]], bufs[br][1 - cur[br]]

            def dst_write(coh, lr0, rt, pv, dst=dst, layer=layer, br=br):
                ov = dst[:, coh, :L].rearrange("p (h w) -> p h w", w=Wp)[
                    :, lr0 + 1:lr0 + 1 + rt, 1:W + 1]
                epi(ov, pv, bia[:, (br * 4 + layer) * 2 + coh])

            conv_tiles_from(nc, lambda g, src=src: src[:, g], dst_write, wr,
                            H, W, row_iter)
            cur[br] = 1 - cur[br]

        def run_final(br):
            _, bfin, NOUT = finals[br]
            cout = cout_c if br == 0 else cout_b
            _final_from(tc, nc, wrts[br], stp,
                        lambda g, br=br: bufs[br][cur[br]][:, g, :L],
                        bfin, cout, NOUT, H, W, list(range(H)), ps_fa, ps_fb)

        cur = dict(start)
        if seq:
            for br in branches:
                load_x(br)
                for layer in range(nlayers):
                    run_layer(br, layer)
                if do_final:
                    run_final(br)
        else:
            for br in branches:
                load_x(br)
            for layer in range(nlayers):
                for br in branches:
                    run_layer(br, layer)
            if do_final:
                for br in branches:
                    run_final(br)


def _level0_stream(tc, nc, x0, acts, wmid, bia, finals, cout_c, cout_b,
                   branches, H, W, nlayers, do_final, conv_tiles_from, epi,
                   ps_fa, ps_fb, sp, op, wp, wrts, stp):
    Hp, Wp = H + 2, W + 2
    SMAX = max(STRIPS0)
    strips = []
    y = 0
    for S in STRIPS0:
        strips.append((y, S))
        y += S
    assert y == H

    if True:
        srcs_br = {br: [x0] + [acts[2 * bi + (i % 2)] for i in range(nlayers)]
                   for bi, br in enumerate(branches)}
        for layer in range(nlayers):
            for br in branches:
                srcs = srcs_br[br]
                wr = _load_wmid(nc, wp, wmid, br, layer)
                src_d, dst_d = srcs[layer], srcs[layer + 1]
                for (y0, S) in strips:
                    xst = sp.tile([128, 2, (SMAX + 2) * Wp + 2], DT_MM,
                                  name=f"xs{br}{layer}{y0}", tag="xst")
                    for gg in range(2):
                        eng = nc.sync if gg == 0 else nc.scalar
                        eng.dma_start(
                            xst[:, gg, :(S + 2) * Wp],
                            src_d[gg, :, y0:y0 + S + 2, :].rearrange("p h w -> p (h w)"))
                    ost = op.tile([128, 2, SMAX, Wp], DT_MM,
                                  name=f"os{br}{layer}{y0}", tag="ost")
                    # zero the junk columns so full-width rows can be DMA'd
                    # out in one contiguous chunk per channel (they land on
                    # the zero borders of the DRAM buffer, which is correct)
                    nc.vector.memset((ost.bitcast(F32) if DT_MM == F32R else ost)[:, :, :S, 0:1], 0.0)
                    nc.vector.memset((ost.bitcast(F32) if DT_MM == F32R else ost)[:, :, :S, Wp - 1:Wp], 0.0)

                    def dst_write(coh, lr0, rt, pv, ost=ost, layer=layer):
                        epi(ost[:, coh, lr0:lr0 + rt, 1:W + 1], pv,
                            bia[:, (br * 4 + layer) * 2 + coh])

                    row_iter = []
                    ly = 0
                    for rt in strip_row_splits(S):
                        row_iter.append((ly, rt))
                        ly += rt
                    conv_tiles_from(nc, lambda g, xst=xst: xst[:, g], dst_write,
                                    wr, H, W, row_iter)
                    for gg in range(2):
                        eng = nc.scalar if gg == 0 else nc.gpsimd
                        eng.dma_start(
                            dst_d[gg, :, y0 + 1:y0 + 1 + S, :],
                            ost[:, gg, :S, :])

        if do_final:
            for br in branches:
                _, bfin, NOUT = finals[br]
                cout = cout_c if br == 0 else cout_b
                src_d = srcs_br[br][nlayers]
                for (y0, S) in strips:
                    xst = sp.tile([128, 2, (SMAX + 2) * Wp + 2], DT_MM,
                                  name=f"fx{br}{y0}", tag="xst")
                    for gg in range(2):
                        eng = nc.sync if gg == 0 else nc.scalar
                        eng.dma_start(
                            xst[:, gg, :(S + 2) * Wp],
                            src_d[gg, :, y0:y0 + S + 2, :].rearrange("p h w -> p (h w)"))

                    def src_flat(g, xst=xst):
                        return xst[:, g]
                    _final_from(tc, nc, wrts[br], stp, src_flat, bfin,
                                cout, NOUT, H, W, list(range(y0, y0 + S)),
                                ps_fa, ps_fb, local_y0=y0)


_finctr = [0]


def _final_from(tc, nc, wrt, stp, src_flat, bfin, cout, NOUT, H, W, rows,
                ps_fa, ps_fb, local_y0=0):
    """Form-A final conv over the given output rows. src_flat(g): flat padded
    AP whose row 0 == padded row local_y0."""
    Wp = W + 2
    nr = 128 // W       # rows packed per PSUM tile via PE column-tiling
    M = nr * W          # = 128
    if NOUT == 720:
        nslices = [(0, 384), (384, 336)]
    else:
        nslices = [(0, NOUT)]
    _finctr[0] += 1
    for i in range(0, len(rows), nr):
        y = rows[i]
        ly = y - local_y0
        stag = stp.tile([128, NOUT], F32, name=f"st{_finctr[0]}_{i}", tag=f"st{NOUT}")
        for (off, nsl) in nslices:
            tag = ("fa" + str(off)) if NOUT == 720 else "fb"
            pool = ps_fa if NOUT == 720 else ps_fb
            p = pool.tile([128, 512], F32, name=f"pf{_finctr[0]}_{i}_{off}", tag=tag)
            k = 0
            for t in range(9):
                dy, dx = t // 3, t % 3
                for g in range(2):
                    for j in range(nr):
                        # each W-wide row goes to its own PE column group;
                        # same-k matmuls for different j run concurrently
                        lhs = src_flat(g)[
                            :, (ly + j + dy) * Wp + dx:(ly + j + dy) * Wp + dx + W]
                        nc.tensor.matmul(
                            p[j * W:(j + 1) * W, :nsl], lhs,
                            wrt[:, t * 2 + g, off:off + nsl],
                            start=(k == 0), stop=(k == 17),
                            tile_position=(0, j * W) if nr > 1 else None)
                    k += 1
            nc.vector.tensor_tensor(stag[:M, off:off + nsl], p[:M, :nsl],
                                    bfin[:M, off:off + nsl], ADD)
        nc.sync.dma_start(cout[y * W:y * W + M, :], stag[:M, :])


# ---------------- host side ----------------

def round_tf32(a: np.ndarray) -> np.ndarray:
    b = np.ascontiguousarray(a, dtype=np.float32).view(np.uint32)
    r = (b + 0x1000) & np.uint32(0xFFFFE000)
    return r.view(np.float32)


def to_mm(a: np.ndarray) -> np.ndarray:
    if USE_BF16:
        return np.asarray(a, np.float32).astype(ml_dtypes.bfloat16)
    return round_tf32(a)


_CACHE = {}


def _get_program():
    if "nc" not in _CACHE:
        _CACHE["nc"] = build_program()
    return _CACHE["nc"]


def _prep_shared(cls_w, cls_b, bbox_w, bbox_b, cls_out_w, cls_out_b,
                 bbox_out_w, bbox_out_b):
    wmid = np.empty((2, 4, 9, 2, 128, 256), np.float32)
    for b, wsrc in ((0, cls_w), (1, bbox_w)):
        for l in range(4):
            for t in range(9):
                dy, dx = t // 3, t % 3
                for g in range(2):
                    wmid[b, l, t, g] = wsrc[l, dy, dx, g * 128:(g + 1) * 128, :]
    wmid = to_mm(wmid.reshape(144, 128, 256))

    bmid = np.empty((2, 4, 2, 128, 1), np.float32)
    for b, bsrc in ((0, cls_b), (1, bbox_b)):
        for l in range(4):
            for coh in range(2):
                bmid[b, l, coh, :, 0] = bsrc[l, coh * 128:(coh + 1) * 128]
    bmid = bmid.reshape(16, 128, 1)

    wcls = np.empty((9, 2, 128, NCLS), np.float32)
    wbox = np.empty((9, 2, 128, NBOX), np.float32)
    for t in range(9):
        dy, dx = t // 3, t % 3
        for g in range(2):
            wcls[t, g] = cls_out_w[dy, dx, g * 128:(g + 1) * 128, :]
            wbox[t, g] = bbox_out_w[dy, dx, g * 128:(g + 1) * 128, :]
    wcls = to_mm(wcls.reshape(18, 128, NCLS))
    wbox = to_mm(wbox.reshape(18, 128, NBOX))

    bclsr = np.broadcast_to(cls_out_b.astype(np.float32), (128, NCLS)).copy()
    bboxr = np.broadcast_to(bbox_out_b.astype(np.float32), (128, NBOX)).copy()
    return dict(wmid=wmid, bmid=bmid, wcls=wcls, bcls=bclsr, wbox=wbox, bbox=bboxr)


def _prep_x(x):
    H, W, _ = x.shape
    dt = ml_dtypes.bfloat16 if USE_BF16 else np.float32
    out = np.zeros((2, 128, H + 2, W + 2), dt)
    xt = to_mm(x).transpose(2, 0, 1).reshape(2, 128, H, W)
    out[:, :, 1:H + 1, 1:W + 1] = xt
    return out


def kernel(x0, x1, x2, cls_w, cls_b, bbox_w, bbox_b,
           cls_out_w, cls_out_b, bbox_out_w, bbox_out_b):
    nc = _get_program()
    cls_w, cls_b, bbox_w, bbox_b = (np.asarray(a, np.float32) for a in
                                    (cls_w, cls_b, bbox_w, bbox_b))
    cls_out_w, cls_out_b, bbox_out_w, bbox_out_b = (
        np.asarray(a, np.float32) for a in
        (cls_out_w, cls_out_b, bbox_out_w, bbox_out_b))
    shared = _prep_shared(cls_w, cls_b, bbox_w, bbox_b, cls_out_w, cls_out_b,
                          bbox_out_w, bbox_out_b)
    in_maps = []
    for n in range(8):
        m = dict(shared)
        m["x0"] = _prep_x(np.asarray(x0)[n])
        m["x1"] = _prep_x(np.asarray(x1)[n])
        m["x2"] = _prep_x(np.asarray(x2)[n])
        in_maps.append(m)
    res = run_bass_kernel_spmd(nc, in_maps, core_ids=list(range(8)),
                               trace=_CACHE.get("trace", False))
    _CACHE["exec_time_ns"] = res.exec_time_ns
    outs = []
    for name, (H, W), NC_ in (("c0", LEVELS[0], NCLS), ("c1", LEVELS[1], NCLS),
                              ("c2", LEVELS[2], NCLS), ("b0", LEVELS[0], NBOX),
                              ("b1", LEVELS[1], NBOX), ("b2", LEVELS[2], NBOX)):
        outs.append(np.stack([res.results[n][name].reshape(H, W, NC_)
                              for n in range(8)]))
    return tuple(outs)
